# revision 9
# baseline (speedup 1.0000x reference)
"""Causal single-head attention (B=4, T=2048, D=1024) on 8 NeuronCores.

Sharding: 2 cores per batch element. Within a batch, core r (r in {0,1})
handles the strided query rows q_glob = 2*j + r (j = 0..1023). The strided
split makes the causal block structure identical on every core (SPMD-safe)
and balances causal work exactly.

Per-core device program (all matmuls bf16 with fp32 PSUM accumulation):
  1. K^T = Wk x^T   [d_out partition-major, 2048 keys]
  2. V   = x Wv^T   [2048 keys partition-major, d_out free]
  3. Q^T = Wq xq^T  [d_out partition-major, 1024 local queries]
  4. per 128-query block: S = Q K^T (fp32 PSUM), additive causal mask on the
     diagonal 512-chunk, exp on ACT (scale 1/32) with accum_out row-sums,
     PE-transpose of P, out2 += P^T V accumulated in PSUM, final per-row
     divide by the softmax denominator.

Host side transposes/bf16-casts inputs and de-interleaves outputs.
"""
import orjson
import numpy as np
import ml_dtypes

import concourse.bass as bass
import concourse.mybir as mybir
import concourse.tile as tile
from concourse import bass_utils
from concourse.masks import make_identity

B, T, D = 4, 2048, 1024
NCORES = 8
P = 128
JQ = T // 2            # local queries per core (1024)
N_QB = JQ // P         # 8 query blocks of 128
N_IT = D // P          # 8 contraction tiles
N_KT = T // P          # 16 key tiles of 128
KC = 512               # key chunk for S matmuls
N_KC = T // KC         # 4
F32 = mybir.dt.float32
BF16 = mybir.dt.bfloat16
SCALE = 1.0 / 32.0     # 1/sqrt(D)
MASK_NEG = -1.0e9

def _split_waits(blocks):
    """The walrus build in this container accepts at most ONE sync-wait per
    instruction; Tile freely emits several. Split extras onto same-engine
    NoOps inserted immediately before the instruction (engine-serial order
    preserves semantics)."""
    n_split = 0
    for blk in blocks:
        insts = blk.get("instructions", [])
        out = []
        for ins in insts:
            si = ins.get("sync_info")
            waits = (si or {}).get("on_wait") or []
            if len(waits) > 1:
                for i, w in enumerate(waits[:-1]):
                    nop = {
                        "engine": ins["engine"],
                        "ins": [],
                        "name": f"{ins['name']}-w{i}",
                        "opcode": "NoOp",
                        "outs": [],
                        "sync_info": {"on_wait": [w], "on_update": []},
                    }
                    if "debug" in ins:
                        nop["debug"] = ins["debug"]
                    out.append(nop)
                    n_split += 1
                si["on_wait"] = [waits[-1]]
            out.append(ins)
        blk["instructions"] = out
        _split_waits(blk.get("blocks", []) or [])
    return n_split


class _Bass(bass.Bass):
    def to_json_bytes(self):
        d = orjson.loads(super().to_json_bytes())
        for f in d["functions"]:
            _split_waits(f["blocks"])
        return orjson.dumps(d)


def n_kc_of(qb: int) -> int:
    # chunks of 512 keys needed by query block qb (covers q_glob < 256*(qb+1))
    return qb // 2 + 1


def build_nc() -> bass.Bass:
    nc = _Bass("TRN2", debug=False, num_devices=NCORES)

    xT = nc.dram_tensor("xT", [D, T], BF16, kind="ExternalInput")
    xqT = nc.dram_tensor("xqT", [D, JQ], BF16, kind="ExternalInput")
    wqT = nc.dram_tensor("wqT", [D, D], BF16, kind="ExternalInput")
    wkT = nc.dram_tensor("wkT", [D, D], BF16, kind="ExternalInput")
    wvT = nc.dram_tensor("wvT", [D, D], BF16, kind="ExternalInput")
    maskadd = nc.dram_tensor("maskadd", [2, P, KC], F32, kind="ExternalInput")
    out = nc.dram_tensor("out", [JQ, D], F32, kind="ExternalOutput")

    with tile.TileContext(nc) as tc:
        with (
            tc.tile_pool(name="big", bufs=1) as big,
            tc.tile_pool(name="wpool", bufs=2) as wpool,
            tc.tile_pool(name="small", bufs=2) as small,
            tc.tile_pool(name="pwork", bufs=3) as pwork,
            tc.tile_pool(name="ptwork", bufs=6) as ptwork,
            tc.tile_pool(name="mm", bufs=2, space="PSUM") as mm,
            tc.tile_pool(name="ptp", bufs=2, space="PSUM") as ptp,
            tc.tile_pool(name="o2p", bufs=2, space="PSUM") as o2p,
        ):
            # ---- constants ----
            ident = big.tile([P, P], BF16, tag="ident")
            make_identity(nc, ident[:])
            masks = big.tile([P, 2, KC], F32, tag="masks")
            # DRAM [2,128,512] -> partition-major per mask
            nc.sync.dma_start(masks[:], maskadd.rearrange("m p f -> p m f"))

            # ---- resident inputs ----
            xt = big.tile([P, N_IT, T], BF16, tag="xt")
            nc.sync.dma_start(xt[:], xT.rearrange("(it p) t -> p it t", p=P))
            xq = big.tile([P, N_IT, JQ], BF16, tag="xq")
            nc.sync.dma_start(xq[:], xqT.rearrange("(it p) t -> p it t", p=P))

            # ---- outputs of the projection phase ----
            kt_sb = big.tile([P, N_IT, T], BF16, tag="kt")
            v_sb = big.tile([P, N_KT, D], BF16, tag="v")
            qt_sb = big.tile([P, N_IT, JQ], BF16, tag="qt")

            def load_w(dram):
                w = wpool.tile([P, N_IT, D], BF16, tag="w")
                nc.sync.dma_start(w[:], dram.rearrange("(it p) o -> p it o", p=P))
                return w

            # ---- K^T projection: K^T[o, t] = sum_i WkT[i,o] * xT[i,t] ----
            wk = load_w(wkT)
            for ot in range(N_IT):
                for tc4 in range(T // KC):
                    acc = mm.tile([P, KC], F32, tag="mm512")
                    for it in range(N_IT):
                        nc.tensor.matmul(
                            acc[:],
                            wk[:, it, ot * P:(ot + 1) * P],
                            xt[:, it, tc4 * KC:(tc4 + 1) * KC],
                            start=(it == 0), stop=(it == N_IT - 1),
                        )
                    nc.vector.tensor_copy(kt_sb[:, ot, tc4 * KC:(tc4 + 1) * KC], acc[:])

            # ---- V projection: V[t, o] = sum_i xT[i,t] * WvT[i,o] ----
            wv = load_w(wvT)
            for tt in range(N_KT):
                for oc in range(D // KC):
                    acc = mm.tile([P, KC], F32, tag="mm512")
                    for it in range(N_IT):
                        nc.tensor.matmul(
                            acc[:],
                            xt[:, it, tt * P:(tt + 1) * P],
                            wv[:, it, oc * KC:(oc + 1) * KC],
                            start=(it == 0), stop=(it == N_IT - 1),
                        )
                    nc.vector.tensor_copy(v_sb[:, tt, oc * KC:(oc + 1) * KC], acc[:])

            # ---- Q^T projection: Q^T[o, j] = sum_i WqT[i,o] * xqT[i,j] ----
            wq = load_w(wqT)
            for ot in range(N_IT):
                for jc in range(JQ // KC):
                    acc = mm.tile([P, KC], F32, tag="mm512")
                    for it in range(N_IT):
                        nc.tensor.matmul(
                            acc[:],
                            wq[:, it, ot * P:(ot + 1) * P],
                            xq[:, it, jc * KC:(jc + 1) * KC],
                            start=(it == 0), stop=(it == N_IT - 1),
                        )
                    nc.vector.tensor_copy(qt_sb[:, ot, jc * KC:(jc + 1) * KC], acc[:])

            # ---- attention, software-pipelined over (qb, kc) ----
            pairs = [(qb, kc) for qb in range(N_QB) for kc in range(n_kc_of(qb))]

            state = {}  # per live qb: dict(out2, lparts)

            def emit_s_phase(qb, kc):
                nkc = n_kc_of(qb)
                if kc == 0:
                    state[qb] = {
                        "out2": o2p.tile([P, D], F32, tag="out2", name="out2"),
                        "lparts": small.tile([P, N_KC], F32, tag="lparts", name="lparts"),
                    }
                s_psum = mm.tile([P, KC], F32, tag="mm512")
                for ot in range(N_IT):
                    nc.tensor.matmul(
                        s_psum[:],
                        qt_sb[:, ot, qb * P:(qb + 1) * P],
                        kt_sb[:, ot, kc * KC:(kc + 1) * KC],
                        start=(ot == 0), stop=(ot == N_IT - 1),
                    )
                if kc == nkc - 1:  # diagonal chunk: additive causal mask
                    nc.vector.tensor_add(s_psum[:], s_psum[:], masks[:, qb % 2, :])
                p_sb = pwork.tile([P, KC], BF16, tag="p")
                nc.scalar.activation(
                    p_sb[:], s_psum[:], mybir.ActivationFunctionType.Exp,
                    scale=SCALE, accum_out=state[qb]["lparts"][:, kc:kc + 1],
                )
                return p_sb

            def emit_pv_phase(qb, kc, p_sb):
                nkc = n_kc_of(qb)
                out2 = state[qb]["out2"]
                for st in range(4):
                    ptps = ptp.tile([P, P], BF16, tag="pt")
                    nc.tensor.transpose(ptps[:], p_sb[:, st * P:(st + 1) * P], ident[:])
                    ptk = ptwork.tile([P, P], BF16, tag="ptk")
                    nc.vector.tensor_copy(ptk[:], ptps[:])
                    kt_idx = kc * 4 + st
                    for oc in range(D // KC):
                        nc.tensor.matmul(
                            out2[:, oc * KC:(oc + 1) * KC],
                            ptk[:],
                            v_sb[:, kt_idx, oc * KC:(oc + 1) * KC],
                            start=(kc == 0 and st == 0),
                            stop=(kc == nkc - 1 and st == 3),
                        )
                if kc == nkc - 1:
                    lparts = state[qb]["lparts"]
                    ltot = small.tile([P, 1], F32, tag="ltot")
                    nc.vector.tensor_reduce(
                        ltot[:], lparts[:, 0:nkc],
                        axis=mybir.AxisListType.X, op=mybir.AluOpType.add,
                    )
                    linv = small.tile([P, 1], F32, tag="linv")
                    nc.vector.reciprocal(linv[:], ltot[:])
                    for oc in range(D // KC):
                        oh = small.tile([P, KC], F32, tag="oh")
                        nc.vector.tensor_scalar_mul(
                            oh[:], out2[:, oc * KC:(oc + 1) * KC], linv[:]
                        )
                        nc.sync.dma_start(
                            out[qb * P:(qb + 1) * P, oc * KC:(oc + 1) * KC], oh[:]
                        )
                    del state[qb]

            prev = None
            for qb, kc in pairs:
                p_sb = emit_s_phase(qb, kc)
                if prev is not None:
                    emit_pv_phase(*prev)
                prev = (qb, kc, p_sb)
            emit_pv_phase(*prev)

    return nc


_NC = None


def _get_nc():
    global _NC
    if _NC is None:
        _NC = build_nc()
    return _NC


def _prep_in_maps(inputs, Wq, Wk, Wv):
    inputs = np.asarray(inputs, dtype=np.float32)
    Wq = np.asarray(Wq, dtype=np.float32)
    Wk = np.asarray(Wk, dtype=np.float32)
    Wv = np.asarray(Wv, dtype=np.float32)

    bf = ml_dtypes.bfloat16
    wqT = np.ascontiguousarray(Wq.T).astype(bf)
    wkT = np.ascontiguousarray(Wk.T).astype(bf)
    wvT = np.ascontiguousarray(Wv.T).astype(bf)

    in_maps = []
    for c in range(NCORES):
        b, r = c // 2, c % 2
        xb = inputs[b]                                  # [T, D]
        xT = np.ascontiguousarray(xb.T).astype(bf)       # [D, T]
        xqT = np.ascontiguousarray(xb[r::2, :].T).astype(bf)  # [D, JQ]
        # additive causal mask for the diagonal 512-chunk:
        # keep (0.0) iff f <= 2p + r + 256*m
        p_idx = np.arange(P)[:, None]
        f_idx = np.arange(KC)[None, :]
        masks = np.empty((2, P, KC), dtype=np.float32)
        for m in range(2):
            keep = f_idx <= 2 * p_idx + r + 256 * m
            masks[m] = np.where(keep, 0.0, MASK_NEG)
        in_maps.append({
            "xT": xT, "xqT": xqT,
            "wqT": wqT, "wkT": wkT, "wvT": wvT,
            "maskadd": masks,
        })
    return in_maps


def _gather(res):
    result = np.empty((B, T, D), dtype=np.float32)
    for c in range(NCORES):
        b, r = c // 2, c % 2
        result[b, r::2, :] = res.results[c]["out"]
    return result


def kernel(inputs, Wq, Wk, Wv):
    in_maps = _prep_in_maps(inputs, Wq, Wk, Wv)
    nc = _get_nc()
    res = bass_utils.run_bass_kernel_spmd(nc, in_maps, core_ids=list(range(NCORES)))
    return _gather(res)


def run_traced(inputs, Wq, Wk, Wv):
    """Like kernel() but with NTFF tracing; returns BassKernelResults
    (exec_time_ns, trace path). For test.py only."""
    in_maps = _prep_in_maps(inputs, Wq, Wk, Wv)
    nc = _get_nc()
    res = bass_utils.run_bass_kernel_spmd(
        nc, in_maps, core_ids=list(range(NCORES)), trace=True
    )
    res.full_output = _gather(res)
    return res


# revision 14
# speedup vs baseline: 1.0770x; 1.0770x over previous
"""Causal single-head attention (B=4, T=2048, D=1024) on 8 NeuronCores.

Sharding: 2 cores per batch element. Within a batch, core r (r in {0,1})
handles the strided query rows q_glob = 2*j + r (j = 0..1023). The strided
split makes the causal block structure identical on every core (SPMD-safe)
and balances causal work exactly.

Per-core device program (all matmuls bf16 with fp32 PSUM accumulation):
  1. K^T = Wk x^T   [d_out partition-major, 2048 keys]
  2. V   = x Wv^T   [2048 keys partition-major, d_out free]
  3. Q^T = Wq xq^T  [d_out partition-major, 1024 local queries]
  4. per 128-query block: S = Q K^T (fp32 PSUM), additive causal mask on the
     diagonal 512-chunk, exp on ACT (scale 1/32) with accum_out row-sums,
     PE-transpose of P, out2 += P^T V accumulated in PSUM, final per-row
     divide by the softmax denominator.

Host side transposes/bf16-casts inputs and de-interleaves outputs.
"""
import orjson
import numpy as np
import ml_dtypes

import concourse.bass as bass
import concourse.mybir as mybir
import concourse.tile as tile
from concourse import bass_utils
from concourse.masks import make_identity

B, T, D = 4, 2048, 1024
NCORES = 8
P = 128
JQ = T // 2            # local queries per core (1024)
N_QB = JQ // P         # 8 query blocks of 128
N_IT = D // P          # 8 contraction tiles
N_KT = T // P          # 16 key tiles of 128
KC = 512               # key chunk for S matmuls
N_KC = T // KC         # 4
F32 = mybir.dt.float32
BF16 = mybir.dt.bfloat16
SCALE = 1.0 / 32.0     # 1/sqrt(D)
MASK_NEG = -1.0e9

def _split_waits(blocks):
    """The walrus build in this container accepts at most ONE sync-wait per
    instruction; Tile freely emits several. Split extras onto same-engine
    NoOps inserted immediately before the instruction (engine-serial order
    preserves semantics)."""
    n_split = 0
    for blk in blocks:
        insts = blk.get("instructions", [])
        out = []
        for ins in insts:
            si = ins.get("sync_info")
            waits = (si or {}).get("on_wait") or []
            if len(waits) > 1:
                for i, w in enumerate(waits[:-1]):
                    nop = {
                        "engine": ins["engine"],
                        "ins": [],
                        "name": f"{ins['name']}-w{i}",
                        "opcode": "NoOp",
                        "outs": [],
                        "sync_info": {"on_wait": [w], "on_update": []},
                    }
                    if "debug" in ins:
                        nop["debug"] = ins["debug"]
                    out.append(nop)
                    n_split += 1
                si["on_wait"] = [waits[-1]]
            out.append(ins)
        blk["instructions"] = out
        _split_waits(blk.get("blocks", []) or [])
    return n_split


class _Bass(bass.Bass):
    def to_json_bytes(self):
        d = orjson.loads(super().to_json_bytes())
        for f in d["functions"]:
            _split_waits(f["blocks"])
        return orjson.dumps(d)


def n_kc_of(qb: int) -> int:
    # chunks of 512 keys needed by query block qb (covers q_glob < 256*(qb+1))
    return qb // 2 + 1


def build_nc() -> bass.Bass:
    nc = _Bass("TRN2", debug=False, num_devices=NCORES)

    xT = nc.dram_tensor("xT", [D, T], BF16, kind="ExternalInput")
    xqT = nc.dram_tensor("xqT", [D, JQ], BF16, kind="ExternalInput")
    wqT = nc.dram_tensor("wqT", [D, D], BF16, kind="ExternalInput")
    wkT = nc.dram_tensor("wkT", [D, D], BF16, kind="ExternalInput")
    wvT = nc.dram_tensor("wvT", [D, D], BF16, kind="ExternalInput")
    maskadd = nc.dram_tensor("maskadd", [2, P, KC], F32, kind="ExternalInput")
    out = nc.dram_tensor("out", [JQ, D], F32, kind="ExternalOutput")

    with tile.TileContext(nc) as tc:
        with (
            tc.tile_pool(name="big", bufs=1) as big,
            tc.tile_pool(name="wpool", bufs=2) as wpool,
            tc.tile_pool(name="small", bufs=2) as small,
            tc.tile_pool(name="pwork", bufs=3) as pwork,
            tc.tile_pool(name="ptwork", bufs=6) as ptwork,
            tc.tile_pool(name="mm", bufs=2, space="PSUM") as mm,
            tc.tile_pool(name="ptp", bufs=2, space="PSUM") as ptp,
            tc.tile_pool(name="o2p", bufs=2, space="PSUM") as o2p,
        ):
            # ---- constants ----
            ident = big.tile([P, P], BF16, tag="ident")
            make_identity(nc, ident[:])

            def load_w(dram):
                w = wpool.tile([P, N_IT, D], BF16, tag="w")
                w_r = dram.rearrange("(it p) o -> it p o", p=P)
                for it in range(N_IT):
                    nc.sync.dma_start(w[:, it, :], w_r[it])
                return w

            # wk first: the opening K-projection groups need all of it
            wk = load_w(wkT)

            # xt chunked tc-major so the tc-outer K loop starts after ~3 MiB
            xt = big.tile([P, N_IT, T], BF16, tag="xt")
            xT_r = xT.rearrange("(it p) t -> it p t", p=P)
            for tc4 in range(T // KC):
                for it in range(N_IT):
                    nc.sync.dma_start(
                        xt[:, it, tc4 * KC:(tc4 + 1) * KC],
                        xT_r[it][:, tc4 * KC:(tc4 + 1) * KC],
                    )

            # ---- outputs of the projection phase ----
            kt_sb = big.tile([P, N_IT, T], BF16, tag="kt")
            v_sb = big.tile([P, N_KT, D], BF16, tag="v")
            qt_sb = big.tile([P, N_IT, JQ], BF16, tag="qt")

            # ---- K^T projection: K^T[o, t] = sum_i WkT[i,o] * xT[i,t] ----
            for tc4 in range(T // KC):
                for ot in range(N_IT):
                    acc = mm.tile([P, KC], F32, tag="mm512")
                    for it in range(N_IT):
                        nc.tensor.matmul(
                            acc[:],
                            wk[:, it, ot * P:(ot + 1) * P],
                            xt[:, it, tc4 * KC:(tc4 + 1) * KC],
                            start=(it == 0), stop=(it == N_IT - 1),
                        )
                    nc.vector.tensor_copy(kt_sb[:, ot, tc4 * KC:(tc4 + 1) * KC], acc[:])

            # ---- V projection: V[t, o] = sum_i xT[i,t] * WvT[i,o] ----
            wv = load_w(wvT)
            xq = big.tile([P, N_IT, JQ], BF16, tag="xq")
            xqT_r = xqT.rearrange("(it p) t -> it p t", p=P)
            for it in range(N_IT):
                nc.sync.dma_start(xq[:, it, :], xqT_r[it])
            for tt in range(N_KT):
                for oc in range(D // KC):
                    acc = mm.tile([P, KC], F32, tag="mm512")
                    for it in range(N_IT):
                        nc.tensor.matmul(
                            acc[:],
                            xt[:, it, tt * P:(tt + 1) * P],
                            wv[:, it, oc * KC:(oc + 1) * KC],
                            start=(it == 0), stop=(it == N_IT - 1),
                        )
                    nc.vector.tensor_copy(v_sb[:, tt, oc * KC:(oc + 1) * KC], acc[:])

            # ---- Q^T projection: Q^T[o, j] = sum_i WqT[i,o] * xqT[i,j] ----
            wq = load_w(wqT)
            for ot in range(N_IT):
                for jc in range(JQ // KC):
                    acc = mm.tile([P, KC], F32, tag="mm512")
                    for it in range(N_IT):
                        nc.tensor.matmul(
                            acc[:],
                            wq[:, it, ot * P:(ot + 1) * P],
                            xq[:, it, jc * KC:(jc + 1) * KC],
                            start=(it == 0), stop=(it == N_IT - 1),
                        )
                    nc.vector.tensor_copy(qt_sb[:, ot, jc * KC:(jc + 1) * KC], acc[:])

            # ---- attention, software-pipelined over (qb, kc) ----
            masks = big.tile([P, 2, KC], F32, tag="masks")
            # DRAM [2,128,512] -> partition-major per mask
            nc.sync.dma_start(masks[:], maskadd.rearrange("m p f -> p m f"))

            pairs = [(qb, kc) for qb in range(N_QB) for kc in range(n_kc_of(qb))]

            state = {}  # per live qb: dict(out2, lparts)

            def width_of(qb, kc):
                # even qb's diagonal chunk only reaches 256 keys in
                return 256 if (qb % 2 == 0 and kc == qb // 2) else KC

            def emit_s_phase(qb, kc):
                nkc = n_kc_of(qb)
                w = width_of(qb, kc)
                if kc == 0:
                    state[qb] = {
                        "out2": o2p.tile([P, D], F32, tag="out2", name="out2"),
                        "lparts": small.tile([P, N_KC], F32, tag="lparts", name="lparts"),
                    }
                s_psum = mm.tile([P, KC], F32, tag="mm512")
                for ot in range(N_IT):
                    nc.tensor.matmul(
                        s_psum[:, :w],
                        qt_sb[:, ot, qb * P:(qb + 1) * P],
                        kt_sb[:, ot, kc * KC:kc * KC + w],
                        start=(ot == 0), stop=(ot == N_IT - 1),
                    )
                if kc == nkc - 1:  # diagonal chunk: additive causal mask
                    nc.vector.tensor_add(s_psum[:, :w], s_psum[:, :w], masks[:, qb % 2, :w])
                p_sb = pwork.tile([P, KC], BF16, tag="p")
                nc.scalar.activation(
                    p_sb[:, :w], s_psum[:, :w], mybir.ActivationFunctionType.Exp,
                    scale=SCALE, accum_out=state[qb]["lparts"][:, kc:kc + 1],
                )
                return p_sb

            def emit_pv_phase(qb, kc, p_sb):
                nkc = n_kc_of(qb)
                w = width_of(qb, kc)
                n_st = w // P
                out2 = state[qb]["out2"]
                for st in range(n_st):
                    ptps = ptp.tile([P, P], BF16, tag="pt")
                    nc.tensor.transpose(ptps[:], p_sb[:, st * P:(st + 1) * P], ident[:])
                    ptk = ptwork.tile([P, P], BF16, tag="ptk")
                    nc.vector.tensor_copy(ptk[:], ptps[:])
                    kt_idx = kc * 4 + st
                    for oc in range(D // KC):
                        nc.tensor.matmul(
                            out2[:, oc * KC:(oc + 1) * KC],
                            ptk[:],
                            v_sb[:, kt_idx, oc * KC:(oc + 1) * KC],
                            start=(kc == 0 and st == 0),
                            stop=(kc == nkc - 1 and st == n_st - 1),
                        )
                if kc == nkc - 1:
                    lparts = state[qb]["lparts"]
                    ltot = small.tile([P, 1], F32, tag="ltot")
                    nc.vector.tensor_reduce(
                        ltot[:], lparts[:, 0:nkc],
                        axis=mybir.AxisListType.X, op=mybir.AluOpType.add,
                    )
                    linv = small.tile([P, 1], F32, tag="linv")
                    nc.vector.reciprocal(linv[:], ltot[:])
                    for oc in range(D // KC):
                        oh = small.tile([P, KC], F32, tag="oh")
                        nc.vector.tensor_scalar_mul(
                            oh[:], out2[:, oc * KC:(oc + 1) * KC], linv[:]
                        )
                        nc.sync.dma_start(
                            out[qb * P:(qb + 1) * P, oc * KC:(oc + 1) * KC], oh[:]
                        )
                    del state[qb]

            prev = None
            for qb, kc in pairs:
                p_sb = emit_s_phase(qb, kc)
                if prev is not None:
                    emit_pv_phase(*prev)
                prev = (qb, kc, p_sb)
            emit_pv_phase(*prev)

    return nc


_NC = None


def _get_nc():
    global _NC
    if _NC is None:
        _NC = build_nc()
    return _NC


def _prep_in_maps(inputs, Wq, Wk, Wv):
    inputs = np.asarray(inputs, dtype=np.float32)
    Wq = np.asarray(Wq, dtype=np.float32)
    Wk = np.asarray(Wk, dtype=np.float32)
    Wv = np.asarray(Wv, dtype=np.float32)

    bf = ml_dtypes.bfloat16
    wqT = np.ascontiguousarray(Wq.T).astype(bf)
    wkT = np.ascontiguousarray(Wk.T).astype(bf)
    wvT = np.ascontiguousarray(Wv.T).astype(bf)

    in_maps = []
    for c in range(NCORES):
        b, r = c // 2, c % 2
        xb = inputs[b]                                  # [T, D]
        xT = np.ascontiguousarray(xb.T).astype(bf)       # [D, T]
        xqT = np.ascontiguousarray(xb[r::2, :].T).astype(bf)  # [D, JQ]
        # additive causal mask for the diagonal 512-chunk:
        # keep (0.0) iff f <= 2p + r + 256*m
        p_idx = np.arange(P)[:, None]
        f_idx = np.arange(KC)[None, :]
        masks = np.empty((2, P, KC), dtype=np.float32)
        for m in range(2):
            keep = f_idx <= 2 * p_idx + r + 256 * m
            masks[m] = np.where(keep, 0.0, MASK_NEG)
        in_maps.append({
            "xT": xT, "xqT": xqT,
            "wqT": wqT, "wkT": wkT, "wvT": wvT,
            "maskadd": masks,
        })
    return in_maps


def _gather(res):
    result = np.empty((B, T, D), dtype=np.float32)
    for c in range(NCORES):
        b, r = c // 2, c % 2
        result[b, r::2, :] = res.results[c]["out"]
    return result


def kernel(inputs, Wq, Wk, Wv):
    in_maps = _prep_in_maps(inputs, Wq, Wk, Wv)
    nc = _get_nc()
    res = bass_utils.run_bass_kernel_spmd(nc, in_maps, core_ids=list(range(NCORES)))
    return _gather(res)


def run_traced(inputs, Wq, Wk, Wv):
    """Like kernel() but with NTFF tracing; returns BassKernelResults
    (exec_time_ns, trace path). For test.py only."""
    in_maps = _prep_in_maps(inputs, Wq, Wk, Wv)
    nc = _get_nc()
    res = bass_utils.run_bass_kernel_spmd(
        nc, in_maps, core_ids=list(range(NCORES)), trace=True
    )
    res.full_output = _gather(res)
    return res


# revision 19
# speedup vs baseline: 1.6585x; 1.5399x over previous
"""Causal single-head attention (B=4, T=2048, D=1024) on 8 NeuronCores.

Sharding: 2 cores per batch element. Within a batch, core r (r in {0,1})
handles the strided query rows q_glob = 2*j + r (j = 0..1023). The strided
split makes the causal block structure identical on every core (SPMD-safe)
and balances causal work exactly.

Per-core device program (all matmuls bf16 with fp32 PSUM accumulation):
  1. K^T = Wk x^T   [d_out partition-major, 2048 keys]
  2. V   = x Wv^T   [2048 keys partition-major, d_out free]
  3. Q^T = Wq xq^T  [d_out partition-major, 1024 local queries]
  4. per 128-query block: S = Q K^T (fp32 PSUM), additive causal mask on the
     diagonal 512-chunk, exp on ACT (scale 1/32) with accum_out row-sums,
     PE-transpose of P, out2 += P^T V accumulated in PSUM, final per-row
     divide by the softmax denominator.

Host side transposes/bf16-casts inputs and de-interleaves outputs.
"""
import orjson
import numpy as np
import ml_dtypes

import concourse.bass as bass
import concourse.mybir as mybir
import concourse.tile as tile
from concourse import bass_utils
from concourse.masks import make_identity

B, T, D = 4, 2048, 1024
NCORES = 8
P = 128
JQ = T // 2            # local queries per core (1024)
N_QB = JQ // P         # 8 query blocks of 128
N_IT = D // P          # 8 contraction tiles
N_KT = T // P          # 16 key tiles of 128
KC = 512               # key chunk for S matmuls
N_KC = T // KC         # 4
F32 = mybir.dt.float32
BF16 = mybir.dt.bfloat16
SCALE = 1.0 / 32.0     # 1/sqrt(D)
MASK_NEG = -1.0e9

def _split_waits(blocks):
    """The walrus build in this container accepts at most ONE sync-wait per
    instruction; Tile freely emits several. Split extras onto same-engine
    NoOps inserted immediately before the instruction (engine-serial order
    preserves semantics)."""
    n_split = 0
    for blk in blocks:
        insts = blk.get("instructions", [])
        out = []
        for ins in insts:
            si = ins.get("sync_info")
            waits = (si or {}).get("on_wait") or []
            if len(waits) > 1:
                for i, w in enumerate(waits[:-1]):
                    nop = {
                        "engine": ins["engine"],
                        "ins": [],
                        "name": f"{ins['name']}-w{i}",
                        "opcode": "NoOp",
                        "outs": [],
                        "sync_info": {"on_wait": [w], "on_update": []},
                    }
                    if "debug" in ins:
                        nop["debug"] = ins["debug"]
                    out.append(nop)
                    n_split += 1
                si["on_wait"] = [waits[-1]]
            out.append(ins)
        blk["instructions"] = out
        _split_waits(blk.get("blocks", []) or [])
    return n_split


class _Bass(bass.Bass):
    def to_json_bytes(self):
        d = orjson.loads(super().to_json_bytes())
        for f in d["functions"]:
            _split_waits(f["blocks"])
        return orjson.dumps(d)


def n_kc_of(qb: int) -> int:
    # chunks of 512 keys needed by query block qb (covers q_glob < 256*(qb+1))
    return qb // 2 + 1


def build_nc() -> bass.Bass:
    nc = _Bass("TRN2", debug=False, num_devices=NCORES)

    # x_halfT: this core's half of the keys (core r of a pair owns keys
    # [r*1024, r*1024+1024)); K/V are computed for the half then pair-AllGathered
    xhT = nc.dram_tensor("xhT", [D, T // 2], BF16, kind="ExternalInput")
    xqT = nc.dram_tensor("xqT", [D, JQ], BF16, kind="ExternalInput")
    wqT = nc.dram_tensor("wqT", [D, D], BF16, kind="ExternalInput")
    wkT = nc.dram_tensor("wkT", [D, D], BF16, kind="ExternalInput")
    wvT = nc.dram_tensor("wvT", [D, D], BF16, kind="ExternalInput")
    maskadd = nc.dram_tensor("maskadd", [2, P, KC], F32, kind="ExternalInput")
    out = nc.dram_tensor("out", [JQ, D], F32, kind="ExternalOutput")

    with tile.TileContext(nc) as tc:
        with (
            tc.tile_pool(name="big", bufs=1) as big,
            tc.tile_pool(name="wpool", bufs=2) as wpool,
            tc.tile_pool(name="small", bufs=2) as small,
            tc.tile_pool(name="pwork", bufs=3) as pwork,
            tc.tile_pool(name="ptwork", bufs=6) as ptwork,
            tc.tile_pool(name="dram", bufs=1, space="DRAM") as dram,
            tc.tile_pool(name="mm", bufs=2, space="PSUM") as mm,
            tc.tile_pool(name="ptp", bufs=2, space="PSUM") as ptp,
            tc.tile_pool(name="o2p", bufs=2, space="PSUM") as o2p,
        ):
            # ---- constants ----
            ident = big.tile([P, P], BF16, tag="ident")
            make_identity(nc, ident[:])

            def load_w(dram):
                w = wpool.tile([P, N_IT, D], BF16, tag="w")
                w_r = dram.rearrange("(it p) o -> it p o", p=P)
                for it in range(N_IT):
                    nc.sync.dma_start(w[:, it, :], w_r[it])
                return w

            # wk first: the opening K-projection groups need all of it
            wk = load_w(wkT)

            # local key half, chunked tc-major so the tc-outer K loop starts early
            TH = T // 2
            xh = big.tile([P, N_IT, TH], BF16, tag="xh")
            xhT_r = xhT.rearrange("(it p) t -> it p t", p=P)
            for tc4 in range(TH // KC):
                for it in range(N_IT):
                    nc.sync.dma_start(
                        xh[:, it, tc4 * KC:(tc4 + 1) * KC],
                        xhT_r[it][:, tc4 * KC:(tc4 + 1) * KC],
                    )

            # ---- resident K^T / V / Q^T for the attention phase ----
            kt_sb = big.tile([P, N_IT, T], BF16, tag="kt")
            v_sb = big.tile([P, N_KT, D], BF16, tag="v")
            qt_sb = big.tile([P, N_IT, JQ], BF16, tag="qt")

            # DRAM bounce buffers for the pair AllGather of local K^T / V
            inb = dram.tile([2, TH, D], BF16, tag="inb")   # [0]=K^T_loc [o,t], [1]=V_loc [t,o]
            outb = dram.tile([2, 2, TH, D], BF16, tag="outb")

            # ---- K^T_local[o, t] = sum_i WkT[i,o] * xhT[i,t] ----
            for tc4 in range(TH // KC):
                for ot in range(N_IT):
                    acc = mm.tile([P, KC], F32, tag="mm512")
                    for it in range(N_IT):
                        nc.tensor.matmul(
                            acc[:],
                            wk[:, it, ot * P:(ot + 1) * P],
                            xh[:, it, tc4 * KC:(tc4 + 1) * KC],
                            start=(it == 0), stop=(it == N_IT - 1),
                        )
                    stg = pwork.tile([P, KC], BF16, tag="kvst", name="stg")
                    nc.vector.tensor_copy(stg[:], acc[:])
                    nc.sync.dma_start(
                        inb[0, ot * P:(ot + 1) * P, tc4 * KC:(tc4 + 1) * KC], stg[:]
                    )

            # ---- V_local[t, o] = sum_i xhT[i,t] * WvT[i,o] ----
            wv = load_w(wvT)
            xq = big.tile([P, N_IT, JQ], BF16, tag="xq")
            xqT_r = xqT.rearrange("(it p) t -> it p t", p=P)
            for it in range(N_IT):
                nc.sync.dma_start(xq[:, it, :], xqT_r[it])
            for tt in range(TH // P):
                for oc in range(D // KC):
                    acc = mm.tile([P, KC], F32, tag="mm512")
                    for it in range(N_IT):
                        nc.tensor.matmul(
                            acc[:],
                            xh[:, it, tt * P:(tt + 1) * P],
                            wv[:, it, oc * KC:(oc + 1) * KC],
                            start=(it == 0), stop=(it == N_IT - 1),
                        )
                    stg = pwork.tile([P, KC], BF16, tag="kvst", name="stg")
                    nc.vector.tensor_copy(stg[:], acc[:])
                    nc.sync.dma_start(
                        inb[1, tt * P:(tt + 1) * P, oc * KC:(oc + 1) * KC], stg[:]
                    )

            # ---- pair AllGather: both halves of K^T and V ----
            nc.gpsimd.collective_compute(
                "AllGather",
                mybir.AluOpType.bypass,
                replica_groups=[[0, 1], [2, 3], [4, 5], [6, 7]],
                ins=[inb.opt()],
                outs=[outb.opt()],
            )
            for h in range(2):
                for ot in range(N_IT):
                    nc.sync.dma_start(
                        kt_sb[:, ot, h * TH:(h + 1) * TH],
                        outb[h, 0, ot * P:(ot + 1) * P, :],
                    )
            for tt16 in range(N_KT):
                h, tl = tt16 // (TH // P), tt16 % (TH // P)
                nc.sync.dma_start(
                    v_sb[:, tt16, :],
                    outb[h, 1, tl * P:(tl + 1) * P, :],
                )

            # ---- Q^T projection: Q^T[o, j] = sum_i WqT[i,o] * xqT[i,j] ----
            wq = load_w(wqT)
            for ot in range(N_IT):
                for jc in range(JQ // KC):
                    acc = mm.tile([P, KC], F32, tag="mm512")
                    for it in range(N_IT):
                        nc.tensor.matmul(
                            acc[:],
                            wq[:, it, ot * P:(ot + 1) * P],
                            xq[:, it, jc * KC:(jc + 1) * KC],
                            start=(it == 0), stop=(it == N_IT - 1),
                        )
                    nc.vector.tensor_copy(qt_sb[:, ot, jc * KC:(jc + 1) * KC], acc[:])

            # ---- attention, software-pipelined over (qb, kc) ----
            masks = big.tile([P, 2, KC], F32, tag="masks")
            # DRAM [2,128,512] -> partition-major per mask
            nc.sync.dma_start(masks[:], maskadd.rearrange("m p f -> p m f"))

            pairs = [(qb, kc) for qb in range(N_QB) for kc in range(n_kc_of(qb))]

            state = {}  # per live qb: dict(out2, lparts)

            def width_of(qb, kc):
                # even qb's diagonal chunk only reaches 256 keys in
                return 256 if (qb % 2 == 0 and kc == qb // 2) else KC

            def emit_s_phase(qb, kc):
                nkc = n_kc_of(qb)
                w = width_of(qb, kc)
                if kc == 0:
                    state[qb] = {
                        "out2": o2p.tile([P, D], F32, tag="out2", name="out2"),
                        "lparts": small.tile([P, N_KC], F32, tag="lparts", name="lparts"),
                    }
                s_psum = mm.tile([P, KC], F32, tag="mm512")
                for ot in range(N_IT):
                    nc.tensor.matmul(
                        s_psum[:, :w],
                        qt_sb[:, ot, qb * P:(qb + 1) * P],
                        kt_sb[:, ot, kc * KC:kc * KC + w],
                        start=(ot == 0), stop=(ot == N_IT - 1),
                    )
                if kc == nkc - 1:  # diagonal chunk: additive causal mask
                    nc.vector.tensor_add(s_psum[:, :w], s_psum[:, :w], masks[:, qb % 2, :w])
                p_sb = pwork.tile([P, KC], BF16, tag="p")
                nc.scalar.activation(
                    p_sb[:, :w], s_psum[:, :w], mybir.ActivationFunctionType.Exp,
                    scale=SCALE, accum_out=state[qb]["lparts"][:, kc:kc + 1],
                )
                return p_sb

            def emit_pv_phase(qb, kc, p_sb):
                nkc = n_kc_of(qb)
                w = width_of(qb, kc)
                n_st = w // P
                out2 = state[qb]["out2"]
                for st in range(n_st):
                    ptps = ptp.tile([P, P], BF16, tag="pt")
                    nc.tensor.transpose(ptps[:], p_sb[:, st * P:(st + 1) * P], ident[:])
                    ptk = ptwork.tile([P, P], BF16, tag="ptk")
                    nc.vector.tensor_copy(ptk[:], ptps[:])
                    kt_idx = kc * 4 + st
                    for oc in range(D // KC):
                        nc.tensor.matmul(
                            out2[:, oc * KC:(oc + 1) * KC],
                            ptk[:],
                            v_sb[:, kt_idx, oc * KC:(oc + 1) * KC],
                            start=(kc == 0 and st == 0),
                            stop=(kc == nkc - 1 and st == n_st - 1),
                        )
                if kc == nkc - 1:
                    lparts = state[qb]["lparts"]
                    ltot = small.tile([P, 1], F32, tag="ltot")
                    nc.vector.tensor_reduce(
                        ltot[:], lparts[:, 0:nkc],
                        axis=mybir.AxisListType.X, op=mybir.AluOpType.add,
                    )
                    linv = small.tile([P, 1], F32, tag="linv")
                    nc.vector.reciprocal(linv[:], ltot[:])
                    for oc in range(D // KC):
                        oh = small.tile([P, KC], F32, tag="oh")
                        nc.vector.tensor_scalar_mul(
                            oh[:], out2[:, oc * KC:(oc + 1) * KC], linv[:]
                        )
                        nc.sync.dma_start(
                            out[qb * P:(qb + 1) * P, oc * KC:(oc + 1) * KC], oh[:]
                        )
                    del state[qb]

            prev = None
            for qb, kc in pairs:
                p_sb = emit_s_phase(qb, kc)
                if prev is not None:
                    emit_pv_phase(*prev)
                prev = (qb, kc, p_sb)
            emit_pv_phase(*prev)

    return nc


_NC = None


def _get_nc():
    global _NC
    if _NC is None:
        _NC = build_nc()
    return _NC


def _prep_in_maps(inputs, Wq, Wk, Wv):
    inputs = np.asarray(inputs, dtype=np.float32)
    Wq = np.asarray(Wq, dtype=np.float32)
    Wk = np.asarray(Wk, dtype=np.float32)
    Wv = np.asarray(Wv, dtype=np.float32)

    bf = ml_dtypes.bfloat16
    wqT = np.ascontiguousarray(Wq.T).astype(bf)
    wkT = np.ascontiguousarray(Wk.T).astype(bf)
    wvT = np.ascontiguousarray(Wv.T).astype(bf)

    in_maps = []
    for c in range(NCORES):
        b, r = c // 2, c % 2
        xb = inputs[b]                                  # [T, D]
        xhT = np.ascontiguousarray(xb[r * (T // 2):(r + 1) * (T // 2), :].T).astype(bf)
        xqT = np.ascontiguousarray(xb[r::2, :].T).astype(bf)  # [D, JQ]
        # additive causal mask for the diagonal 512-chunk:
        # keep (0.0) iff f <= 2p + r + 256*m
        p_idx = np.arange(P)[:, None]
        f_idx = np.arange(KC)[None, :]
        masks = np.empty((2, P, KC), dtype=np.float32)
        for m in range(2):
            keep = f_idx <= 2 * p_idx + r + 256 * m
            masks[m] = np.where(keep, 0.0, MASK_NEG)
        in_maps.append({
            "xhT": xhT, "xqT": xqT,
            "wqT": wqT, "wkT": wkT, "wvT": wvT,
            "maskadd": masks,
        })
    return in_maps


def _gather(res):
    result = np.empty((B, T, D), dtype=np.float32)
    for c in range(NCORES):
        b, r = c // 2, c % 2
        result[b, r::2, :] = res.results[c]["out"]
    return result


def kernel(inputs, Wq, Wk, Wv):
    in_maps = _prep_in_maps(inputs, Wq, Wk, Wv)
    nc = _get_nc()
    res = bass_utils.run_bass_kernel_spmd(nc, in_maps, core_ids=list(range(NCORES)))
    return _gather(res)


def run_traced(inputs, Wq, Wk, Wv):
    """Like kernel() but with NTFF tracing; returns BassKernelResults
    (exec_time_ns, trace path). For test.py only."""
    in_maps = _prep_in_maps(inputs, Wq, Wk, Wv)
    nc = _get_nc()
    res = bass_utils.run_bass_kernel_spmd(
        nc, in_maps, core_ids=list(range(NCORES)), trace=True
    )
    res.full_output = _gather(res)
    return res
